# revision 1
# baseline (speedup 1.0000x reference)
"""AttackHead Trainium2 kernel: builder + host prep/post.

Per-core program (NB batches):
  - per-node first-layer precompute (PE) -> row-major U_src/U_tgt tables,
    staged through SBUF and written to a DRAM scratch
  - HBM row-major dma_gather assembles per-edge U rows (edge-major)
  - DVE add (s+t), PE transpose (identity-stationary) into PSUM,
    ACT relu+bias (per-partition, feature-major) -> z^T tiles
  - PE second layer [W2|A2 fused, 65 outputs] -> PSUM [65, 512]
  - penalty path: HBM row-major dma_gather of per-node scalar rows,
    wrapped [128, E/128] flag math, PE transpose + DMA rearrange,
    penalties/mask fused into PSUM + one scalar_tensor_tensor epilogue.
"""

import sys

sys.path.insert(0, "/opt/trn_rl_repo")

from contextlib import ExitStack

import ml_dtypes
import numpy as np

import concourse.bacc as bacc
import concourse.bass as bass
import concourse.mybir as mybir
import concourse.tile as tile
from concourse import library_config
from concourse.tile import add_dep_helper

BF16 = ml_dtypes.bfloat16

N, D, E, H, M = 4096, 256, 8192, 256, 64
UF = 384  # U-row features: 256 h + 128 g
FC = 3  # feature chunks of 128
ZGI = 512  # z-gather chunk (edges per gather)
PC = 1024  # pen-gather chunk (edges per gather)
MC = 512  # second-layer matmul chunk (edges)


def prep_shared(W1, b1, W2, b2, A1, a1, A2, a2):
    """Weight/bias tensors shared by all cores (numpy, layout only)."""
    Wcat = np.concatenate([W1, A1], axis=1)  # [512, 384]
    wboth = np.concatenate([Wcat[:256], Wcat[256:]], axis=1)  # [256, 768]
    wboth = np.ascontiguousarray(wboth.reshape(2, 128, 768)).astype(BF16)

    # output columns: 0..63 = army logits, 64 = edge logit
    wo = np.zeros((384, 65), np.float32)
    wo[256:384, 0:64] = A2
    wo[0:256, 64] = W2[:, 0]
    wo = np.ascontiguousarray(wo.reshape(3, 128, 65).transpose(1, 0, 2)).astype(BF16)

    biasvec = np.zeros((128, 1), np.float32)
    biasvec[0:64, 0] = a2
    biasvec[64, 0] = float(b2[0]) - 100.0  # -100: eq-penalty shift

    b1a1 = np.zeros((128, 3), np.float32)  # relu bias per f-chunk
    b1a1[:, 0] = b1[0:128]
    b1a1[:, 1] = b1[128:256]
    b1a1[:, 2] = a1

    mvec2 = (np.arange(128, dtype=np.float32) + 1.0).reshape(128, 1)  # p -> m+1
    mvec2[64, 0] = -1.0e30  # row 64 (edge logit): mask always 0

    ident = np.eye(128, dtype=np.float32).astype(BF16)
    return dict(wboth=wboth, wo=wo, biasvec=biasvec, b1a1=b1a1, mvec=mvec2,
                ident=ident)


def _wrap16(idx):
    """[NB, E] -> int16 [NB, 128, E//16]: idx j at (j%16, j//16), x8 groups."""
    nb = idx.shape[0]
    w = idx.reshape(nb, E // 16, 16).transpose(0, 2, 1)
    return np.ascontiguousarray(np.tile(w, (1, 8, 1))).astype(np.int16)


def prep_core(node_embeddings, army_counts, action_edges):
    """Per-core input shard prep (numpy, layout/dtype only)."""
    nb = node_embeddings.shape[0]
    nodeT = np.ascontiguousarray(node_embeddings.transpose(0, 2, 1)).reshape(
        nb, 2, 128, N
    ).astype(BF16)

    src = np.clip(action_edges[..., 0], 0, N - 1)
    tgt = np.clip(action_edges[..., 1], 0, N - 1)
    src_w = _wrap16(src)
    tgt_w = _wrap16(tgt)

    ids = np.arange(N, dtype=np.float32)
    pent = np.zeros((nb, N, 128), np.float32)
    pent[:, :, 0] = army_counts  # a
    pent[:, :, 1] = 3.0 * army_counts + 2.0  # 3a + 2
    pent[:, :, 2] = ids[None, :] // 64  # hi
    pent[:, :, 3] = ids[None, :] % 64  # lo
    pent = pent.astype(BF16)

    return dict(node_t=nodeT, src_w=src_w, tgt_w=tgt_w, pent=pent)


def build_nc(NB=8, num_devices=8):
    f32 = mybir.dt.float32
    bf16 = mybir.dt.bfloat16
    i16 = mybir.dt.int16
    Relu = mybir.ActivationFunctionType.Relu
    Op = mybir.AluOpType

    nc = bacc.Bacc("TRN2", target_bir_lowering=False, debug=False,
                   num_devices=num_devices)

    node_d = nc.dram_tensor("node_t", [NB, 2, 128, N], bf16, kind="ExternalInput").ap()
    wboth_d = nc.dram_tensor("wboth", [2, 128, 768], bf16, kind="ExternalInput").ap()
    wo_d = nc.dram_tensor("wo", [128, 3, 65], bf16, kind="ExternalInput").ap()
    bias_d = nc.dram_tensor("biasvec", [128, 1], f32, kind="ExternalInput").ap()
    b1a1_d = nc.dram_tensor("b1a1", [128, 3], f32, kind="ExternalInput").ap()
    mvec_d = nc.dram_tensor("mvec", [128, 1], f32, kind="ExternalInput").ap()
    pent_d = nc.dram_tensor("pent", [NB, N, 128], bf16, kind="ExternalInput").ap()
    srcw_d = nc.dram_tensor("src_w", [NB, 128, E // 16], i16, kind="ExternalInput").ap()
    tgtw_d = nc.dram_tensor("tgt_w", [NB, 128, E // 16], i16, kind="ExternalInput").ap()
    ident_d = nc.dram_tensor("ident", [128, 128], bf16, kind="ExternalInput").ap()

    edge_d = nc.dram_tensor("edge_out", [NB, E], f32, kind="ExternalOutput").ap()
    army_d = nc.dram_tensor("army_out", [NB, M, E], f32, kind="ExternalOutput").ap()

    # DRAM scratch for the per-batch U tables (double-buffered across batches)
    uscr_d = nc.dram_tensor("uscr", [2, 2, N, UF], bf16).ap()

    gathers = []

    with tile.TileContext(nc) as tc, ExitStack() as ctx:
        const_p = ctx.enter_context(tc.tile_pool(name="const", bufs=1))
        node_p = ctx.enter_context(tc.tile_pool(name="node", bufs=2))
        ustg_p = ctx.enter_context(tc.tile_pool(name="ustg", bufs=4))
        idx_p = ctx.enter_context(tc.tile_pool(name="idx", bufs=2))
        zg_p = ctx.enter_context(tc.tile_pool(name="zg", bufs=3))
        zs_p = ctx.enter_context(tc.tile_pool(name="zs", bufs=3))
        z_p = ctx.enter_context(tc.tile_pool(name="z", bufs=3))
        pen_p = ctx.enter_context(tc.tile_pool(name="pen", bufs=2))
        wr_p = ctx.enter_context(tc.tile_pool(name="wrap", bufs=4))
        small_p = ctx.enter_context(tc.tile_pool(name="small", bufs=2))
        ptile_p = ctx.enter_context(tc.tile_pool(name="ptile", bufs=2))
        ost_p = ctx.enter_context(tc.tile_pool(name="ost", bufs=4))
        psu_p = ctx.enter_context(tc.tile_pool(name="psum_u", bufs=1, space="PSUM"))
        psz_p = ctx.enter_context(tc.tile_pool(name="psum_z", bufs=1, space="PSUM"))
        pso_p = ctx.enter_context(tc.tile_pool(name="psum_o", bufs=1, space="PSUM"))
        psa_p = ctx.enter_context(tc.tile_pool(name="psum_sa", bufs=1, space="PSUM"))
        pst_p = ctx.enter_context(tc.tile_pool(name="psum_t", bufs=1, space="PSUM"))

        lib = nc.gpsimd.load_library(library_config.mlp)

        wboth_sb = const_p.tile([128, 2, 768], bf16)
        nc.sync.dma_start(out=wboth_sb[:], in_=wboth_d.rearrange("c p f -> p c f"))
        wo_sb = const_p.tile([128, 3, 65], bf16)
        nc.sync.dma_start(out=wo_sb[:], in_=wo_d[:])
        bias_sb = const_p.tile([128, 1], f32)
        nc.sync.dma_start(out=bias_sb[:], in_=bias_d[:])
        b1a1_sb = const_p.tile([128, 3], f32)
        nc.sync.dma_start(out=b1a1_sb[:], in_=b1a1_d[:])
        mvec_sb = const_p.tile([128, 1], f32)
        nc.sync.dma_start(out=mvec_sb[:], in_=mvec_d[:])
        ident_sb = const_p.tile([128, 128], bf16)
        nc.sync.dma_start(out=ident_sb[:], in_=ident_d[:])
        ones_sb = const_p.tile([1, 65], bf16)
        nc.vector.memset(ones_sb[:], 1.0)

        for b in range(NB):
            usc = uscr_d[b % 2]  # [2, N, UF] dram scratch for this batch

            # ---- load nodeT + idx tiles --------------------------------
            ndt = node_p.tile([128, 2, N], bf16, tag="ndt")
            nc.sync.dma_start(out=ndt[:], in_=node_d[b].rearrange("c p n -> p c n"))
            swt = idx_p.tile([128, E // 16], i16, tag="swt")
            nc.sync.dma_start(out=swt[:], in_=srcw_d[b])
            twt = idx_p.tile([128, E // 16], i16, tag="twt")
            nc.sync.dma_start(out=twt[:], in_=tgtw_d[b])

            # ---- per-node precompute -> DRAM U tables ------------------
            for nch in range(N // 128):
                pss = psu_p.tile([128, UF], f32, tag="pss")
                pst = psu_p.tile([128, UF], f32, tag="pst")
                for dc in range(2):
                    lhs = ndt[:, dc, nch * 128:(nch + 1) * 128]
                    nc.tensor.matmul(out=pss[:], lhsT=lhs, rhs=wboth_sb[:, dc, 0:UF],
                                     start=(dc == 0), stop=(dc == 1))
                    nc.tensor.matmul(out=pst[:], lhsT=lhs, rhs=wboth_sb[:, dc, UF:2 * UF],
                                     start=(dc == 0), stop=(dc == 1))
                stg_s = ustg_p.tile([128, UF], bf16, tag="stg_s")
                stg_t = ustg_p.tile([128, UF], bf16, tag="stg_t")
                nc.vector.tensor_copy(out=stg_s[:], in_=pss[:])
                nc.scalar.activation(out=stg_t[:], in_=pst[:],
                                     func=mybir.ActivationFunctionType.Copy)
                nc.sync.dma_start(out=usc[0, nch * 128:(nch + 1) * 128, :],
                                  in_=stg_s[:])
                nc.sync.dma_start(out=usc[1, nch * 128:(nch + 1) * 128, :],
                                  in_=stg_t[:])

            # ---- pen path ---------------------------------------------
            packed = small_p.tile([128, 128], bf16, tag="packed")
            for pcc in range(E // PC):
                nj = PC // 128
                pgs = pen_p.tile([128, nj, 128], bf16, tag="pgs")
                pgt = pen_p.tile([128, nj, 128], bf16, tag="pgt")
                g1 = nc.gpsimd.dma_gather(
                    pgs[:], pent_d[b], swt[:, pcc * (PC // 16):(pcc + 1) * (PC // 16)],
                    PC, PC, 128, elem_step=128)
                g2 = nc.gpsimd.dma_gather(
                    pgt[:], pent_d[b], twt[:, pcc * (PC // 16):(pcc + 1) * (PC // 16)],
                    PC, PC, 128, elem_step=128)
                gathers += [g1, g2]
                w_ = wr_p.tile([128, nj], f32, tag="w")
                q_ = wr_p.tile([128, nj], f32, tag="q")
                pb_ = wr_p.tile([128, nj], f32, tag="pb")
                dh_ = wr_p.tile([128, nj], f32, tag="dh")
                dl_ = wr_p.tile([128, nj], f32, tag="dl")
                zz_ = wr_p.tile([128, nj], f32, tag="zz")
                pe_ = wr_p.tile([128, nj], f32, tag="pe")
                nc.vector.tensor_tensor(out=w_[:], in0=pgs[:, :, 1], in1=pgt[:, :, 0],
                                        op=Op.subtract)
                nc.vector.tensor_tensor(out=q_[:], in0=w_[:], in1=pgs[:, :, 0], op=Op.min)
                nc.vector.tensor_scalar(out=pb_[:], in0=q_[:], scalar1=2.5, scalar2=-1.0,
                                        op0=Op.is_le, op1=Op.mult)
                nc.vector.tensor_tensor(out=dh_[:], in0=pgs[:, :, 2], in1=pgt[:, :, 2],
                                        op=Op.subtract)
                nc.vector.tensor_tensor(out=dl_[:], in0=pgs[:, :, 3], in1=pgt[:, :, 3],
                                        op=Op.subtract)
                nc.vector.tensor_tensor(out=dh_[:], in0=dh_[:], in1=dh_[:], op=Op.mult)
                nc.vector.tensor_tensor(out=dl_[:], in0=dl_[:], in1=dl_[:], op=Op.mult)
                nc.vector.tensor_tensor(out=zz_[:], in0=dh_[:], in1=dl_[:], op=Op.add)
                nc.vector.tensor_scalar(out=pe_[:], in0=zz_[:], scalar1=1.0, scalar2=100.0,
                                        op0=Op.min, op1=Op.mult)
                nc.vector.tensor_tensor(out=packed[:, pcc * nj:(pcc + 1) * nj],
                                        in0=pb_[:], in1=pe_[:], op=Op.add)
                nc.vector.tensor_copy(out=packed[:, 64 + pcc * nj:64 + (pcc + 1) * nj],
                                      in_=pgs[:, :, 0])

            ptp = pst_p.tile([128, 128], bf16, tag="ptp")
            nc.tensor.transpose(out=ptp[:], in_=packed[:], identity=ident_sb[:])
            pts = small_p.tile([128, 128], bf16, tag="pts")
            nc.vector.tensor_copy(out=pts[:], in_=ptp[:])

            # rows tile: partition 0 = sa (natural order), 64 = edge penalty
            rows = ptile_p.tile([65, E], bf16, tag="rows")
            nc.sync.dma_start(out=rows[64:65, :], in_=pts[0:64, :])
            nc.sync.dma_start(out=rows[0:1, :], in_=pts[64:128, :])

            # ---- z path: gathers + add + transpose + relu + layer 2 ----
            for mcc in range(E // MC):  # 512-edge groups
                gbase = mcc * (MC // ZGI)
                zt0 = psz_p.tile([128, MC], bf16, tag="zt0")
                zt1 = psz_p.tile([128, MC], bf16, tag="zt1")
                zt2 = psz_p.tile([128, MC], bf16, tag="zt2")
                zt_ps = [zt0, zt1, zt2]
                zsum_list = []
                for gi in range(MC // ZGI):
                    gcc = gbase + gi
                    njz = ZGI // 128
                    gs = zg_p.tile([128, njz, UF], bf16, tag="gs")
                    gt = zg_p.tile([128, njz, UF], bf16, tag="gt")
                    g1 = nc.gpsimd.dma_gather(
                        gs[:], usc[0], swt[:, gcc * (ZGI // 16):(gcc + 1) * (ZGI // 16)],
                        ZGI, ZGI, UF, elem_step=UF)
                    g2 = nc.gpsimd.dma_gather(
                        gt[:], usc[1], twt[:, gcc * (ZGI // 16):(gcc + 1) * (ZGI // 16)],
                        ZGI, ZGI, UF, elem_step=UF)
                    gathers += [g1, g2]
                    zsum = zs_p.tile([128, njz, UF], bf16, tag="zsum")
                    nc.vector.tensor_tensor(
                        out=zsum[:].rearrange("p a b -> p (a b)"),
                        in0=gs[:].rearrange("p a b -> p (a b)"),
                        in1=gt[:].rearrange("p a b -> p (a b)"), op=Op.add)
                    zsum_list.append(zsum)
                # transpose 128-edge slots into feature-major PSUM tiles
                for gi, zsum in enumerate(zsum_list):
                    for j in range(ZGI // 128):
                        eoff = gi * ZGI + j * 128 - 0
                        for c in range(FC):
                            nc.tensor.transpose(
                                out=zt_ps[c][:, eoff:eoff + 128],
                                in_=zsum[:, j, c * 128:(c + 1) * 128],
                                identity=ident_sb[:])
                zsb = z_p.tile([128, FC, MC], bf16, tag="zsb")
                for c in range(FC):
                    nc.scalar.activation(out=zsb[:, c, :], in_=zt_ps[c][:],
                                         func=Relu, bias=b1a1_sb[:, c:c + 1])
                ps = pso_p.tile([65, MC], f32, tag="ps")
                for c in range(FC):
                    nc.tensor.matmul(out=ps[:], lhsT=wo_sb[:, c, :],
                                     rhs=zsb[:, c, :],
                                     start=(c == 0), stop=False)
                nc.tensor.matmul(out=ps[:], lhsT=ident_sb[64:65, 0:65],
                                 rhs=rows[64:65, mcc * MC:(mcc + 1) * MC],
                                 start=False, stop=True)
                pssa = psa_p.tile([65, MC], f32, tag="pssa")
                nc.tensor.matmul(out=pssa[:], lhsT=ones_sb[:],
                                 rhs=rows[0:1, mcc * MC:(mcc + 1) * MC],
                                 start=True, stop=True)
                amask = ost_p.tile([65, MC], f32, tag="amask")
                nc.vector.tensor_scalar(
                    out=amask[:], in0=pssa[:],
                    scalar1=mvec_sb[0:65], scalar2=-1.0e9,
                    op0=Op.is_lt, op1=Op.mult)
                ost = ost_p.tile([65, MC], f32, tag="ost")
                nc.vector.scalar_tensor_tensor(
                    out=ost[:], in0=ps[:], scalar=bias_sb[0:65],
                    in1=amask[:], op0=Op.add, op1=Op.add)
                nc.sync.dma_start(out=edge_d[b, mcc * MC:(mcc + 1) * MC],
                                  in_=ost[64:65, :])
                nc.sync.dma_start(out=army_d[b, :, mcc * MC:(mcc + 1) * MC],
                                  in_=ost[0:64, :])

        for g in gathers:
            add_dep_helper(g.ins, lib.ins, reason="gather needs mlp library loaded")

    nc.compile()
    return nc


def postprocess(results, n_cores=8, nb=8):
    edge = np.concatenate([r["edge_out"] for r in results], axis=0)
    army = np.concatenate([r["army_out"] for r in results], axis=0)
    army = np.ascontiguousarray(army.transpose(0, 2, 1))
    return edge.astype(np.float32), army.astype(np.float32)


# ----------------------------------------------------------------------------
# Self-contained kernel entry point: takes FULL inputs, shards over 8 cores,
# runs the Bass kernel, gathers and returns the full outputs.
# ----------------------------------------------------------------------------
from concourse.bass_utils import run_bass_kernel_spmd

_NC_CACHE = {}


def _get_nc():
    if "nc" not in _NC_CACHE:
        _NC_CACHE["nc"] = build_nc(NB=8, num_devices=8)
    return _NC_CACHE["nc"]


def kernel(node_embeddings, army_counts, W1, b1, W2, b2, A1, a1, A2, a2,
           action_edges):
    node_embeddings = np.asarray(node_embeddings, dtype=np.float32)
    army_counts = np.asarray(army_counts, dtype=np.float32)
    action_edges = np.asarray(action_edges, dtype=np.int32)
    W1 = np.asarray(W1, dtype=np.float32); b1 = np.asarray(b1, dtype=np.float32)
    W2 = np.asarray(W2, dtype=np.float32); b2 = np.asarray(b2, dtype=np.float32)
    A1 = np.asarray(A1, dtype=np.float32); a1 = np.asarray(a1, dtype=np.float32)
    A2 = np.asarray(A2, dtype=np.float32); a2 = np.asarray(a2, dtype=np.float32)

    nc = _get_nc()
    shared = prep_shared(W1, b1, W2, b2, A1, a1, A2, a2)
    in_maps = []
    for c in range(8):
        sl = slice(c * 8, (c + 1) * 8)
        core = prep_core(node_embeddings[sl], army_counts[sl], action_edges[sl])
        in_maps.append({**shared, **core})
    res = run_bass_kernel_spmd(nc, in_maps, list(range(8)))
    return postprocess(res.results)
